# revision 11
# baseline (speedup 1.0000x reference)
"""CrossAttentionNoGate, head-sharded over 8 TRN2 cores (v2).

Core h computes head h for all 4 batches:
  Q_T[b] [32,2048] = (x_q[b] @ wq_h*scale).T   (packed at partitions 32b)
  K_T[b] [32,2048] = (x_kv[b] @ wk_h).T
  V_T[b] [32,2048]  -> PE-transposed per 128-slab -> V_aug[b][t] [128,33]
                      = [mask | V*mask]
  per (qc, pr, t): s01 [128,1024] PSUM = [bias+K^TQ(b_lo) | bias+K^TQ(b_hi)]
      (bias added by PE identity-inject for BOTH batches)
  p = exp(s01)  (single ACT op over 2 banks)
  o_both accumulates V_aug^T p for both batches (one PSUM bank, offsets 0/64)
  normalize by denominator row -> bf16 -> DRAM send buffers
Two bf16 AllToAlls (after qc1 and qc3), dest core d = 2b + (qc&1),
column = 512*(qc>>1).  Final: out rows = otl.T @ w_o + b_o  (bf16 matmuls).
"""
from contextlib import ExitStack

import numpy as np

import concourse.bass as bass
import concourse.tile as tile
from concourse import bacc, mybir

F32 = mybir.dt.float32
F32R = mybir.dt.float32r
BF16 = mybir.dt.bfloat16
AF = mybir.ActivationFunctionType

B, Q, KV, C_Q = 4, 2048, 2048, 256
CH = 32
N_CORES = 8
QC = 512
N_QC = Q // QC        # 4
N_SLAB = KV // 128    # 16
SCALE = 1.0 / np.sqrt(CH)


def build():
    nc = bacc.Bacc("TRN2", target_bir_lowering=False, debug=False, num_devices=N_CORES)

    x_qT = nc.dram_tensor("x_qt", [B, C_Q, Q], F32, kind="ExternalInput").ap()
    x_kvT = nc.dram_tensor("x_kvt", [B, C_Q, KV], F32, kind="ExternalInput").ap()
    # [cc(2)][wq|wk|wv][CH] packed, wq pre-scaled by 1/sqrt(CH)
    w3 = nc.dram_tensor("w3", [C_Q // 2, 6 * CH], BF16, kind="ExternalInput").ap()
    bias_t = nc.dram_tensor("bias_t", [KV, Q], F32, kind="ExternalInput").ap()
    mask_p = nc.dram_tensor("mask_p", [B, 128, N_SLAB], F32, kind="ExternalInput").ap()
    ident = nc.dram_tensor("ident", [128, 128], BF16, kind="ExternalInput").ap()
    ident32 = nc.dram_tensor("ident32", [32, 32], F32, kind="ExternalInput").ap()
    ones_bf = nc.dram_tensor("ones_bf", [1, 128], BF16, kind="ExternalInput").ap()
    wo_bf = nc.dram_tensor("wo_bf", [C_Q // 2, 2 * C_Q], BF16, kind="ExternalInput").ap()
    bo_bf = nc.dram_tensor("bo_bf", [1, C_Q], BF16, kind="ExternalInput").ap()

    out = nc.dram_tensor("out", [B * Q // N_CORES, C_Q], F32, kind="ExternalOutput").ap()

    with tile.TileContext(nc) as tc, ExitStack() as st:
        constp = st.enter_context(tc.tile_pool(name="const", bufs=1))
        persist = st.enter_context(tc.tile_pool(name="persist", bufs=1))
        dramp = st.enter_context(tc.tile_pool(name="dram", bufs=1, space="DRAM"))
        biasp = st.enter_context(tc.tile_pool(name="biasp", bufs=4))
        ebp = st.enter_context(tc.tile_pool(name="ebp", bufs=16))
        s_ps = st.enter_context(tc.tile_pool(name="s_ps", bufs=2, space="PSUM"))
        o_ps = st.enter_context(tc.tile_pool(name="o_ps", bufs=1, space="PSUM"))
        ppool = st.enter_context(tc.tile_pool(name="ppool", bufs=3))
        normp = st.enter_context(tc.tile_pool(name="normp", bufs=2))
        proj_st = st.enter_context(ExitStack())
        xpool = proj_st.enter_context(tc.tile_pool(name="xpool", bufs=2))
        tmpp = proj_st.enter_context(tc.tile_pool(name="tmpp", bufs=2))
        pj_ps = proj_st.enter_context(tc.tile_pool(name="pj_ps", bufs=2, space="PSUM"))

        # ---- constants ----
        id_sb = constp.tile([128, 128], BF16)
        nc.sync.dma_start(id_sb[:], ident[:])
        id32_sb = constp.tile([32, 32], F32)
        nc.sync.dma_start(id32_sb[:], ident32[:])
        w3_sb = constp.tile([128, 6 * CH], BF16)
        nc.sync.dma_start(w3_sb[:], w3[:])
        mask_sb = constp.tile([128, B * N_SLAB], F32)
        for b in range(B):
            nc.sync.dma_start(mask_sb[:, b * N_SLAB:(b + 1) * N_SLAB], mask_p[b])
        ones_sb = constp.tile([1, 128], BF16)
        nc.sync.dma_start(ones_sb[:], ones_bf[:])
        wo_sb = constp.tile([128, 2 * C_Q], BF16)
        nc.sync.dma_start(wo_sb[:], wo_bf[:])
        bo_sb = constp.tile([1, C_Q], BF16)
        nc.sync.dma_start(bo_sb[:], bo_bf[:])

        # persistent activations
        qt_sb = persist.tile([128, Q], BF16)
        kt_sb = persist.tile([128, Q], BF16)
        vaug_sb = persist.tile([128, B * N_SLAB * 33], BF16)
        # split exchange buffers: ex 0 covers qc0+qc1, ex 1 covers qc2+qc3
        ot_send = [dramp.tile([N_CORES, CH, QC], BF16, name=f"ot_send{e}")
                   for e in range(2)]
        ot_recv = [dramp.tile([N_CORES, CH, QC], BF16, name=f"ot_recv{e}")
                   for e in range(2)]

        def load_bias(qcp):
            tiles = []
            for t in range(N_SLAB):
                bt = biasp.tile([128, 2 * QC], BF16, tag="bias",
                                name=f"bias_{qcp}_{t}")
                nc.gpsimd.dma_start(
                    bt[:], bias_t[t * 128:(t + 1) * 128,
                                  qcp * 2 * QC:(qcp + 1) * 2 * QC])
                eb = ebp.tile([128, 2 * QC], BF16, tag="eb", name=f"eb_{qcp}_{t}")
                nc.scalar.activation(eb[:], bt[:], AF.Exp)
                tiles.append(eb)
            return tiles

        def proj(b):
            xq = xpool.tile([128, 2 * Q], BF16, tag="xq", name=f"xq{b}")
            xkv = xpool.tile([128, 2 * KV], BF16, tag="xkv", name=f"xkv{b}")
            for cc in range(2):
                nc.gpsimd.dma_start(xq[:, cc * Q:(cc + 1) * Q],
                                    x_qT[b, cc * 128:(cc + 1) * 128, :])
                nc.gpsimd.dma_start(xkv[:, cc * KV:(cc + 1) * KV],
                                    x_kvT[b, cc * 128:(cc + 1) * 128, :])
            # q/k projections -> [32, 512] psum stripes -> qt/kt rows 32b
            for qc in range(N_QC):
                pq = pj_ps.tile([32, QC], F32, tag="pj", name=f"pq{b}_{qc}")
                pk = pj_ps.tile([32, QC], F32, tag="pj", name=f"pk{b}_{qc}")
                for cc in range(2):
                    nc.tensor.matmul(
                        pq[:], w3_sb[:, cc * 96:cc * 96 + CH],
                        xq[:, cc * Q + qc * QC: cc * Q + (qc + 1) * QC],
                        start=(cc == 0), stop=(cc == 1))
                    nc.tensor.matmul(
                        pk[:], w3_sb[:, cc * 96 + CH:cc * 96 + 2 * CH],
                        xkv[:, cc * KV + qc * QC: cc * KV + (qc + 1) * QC],
                        start=(cc == 0), stop=(cc == 1))
                tmpq = tmpp.tile([32, QC], BF16, tag="tq", name=f"tq{b}_{qc}")
                tmpk = tmpp.tile([32, QC], BF16, tag="tk", name=f"tk{b}_{qc}")
                nc.vector.tensor_copy(tmpq[:], pq[:])
                nc.vector.tensor_copy(tmpk[:], pk[:])
                nc.sync.dma_start(
                    qt_sb[32 * b:32 * (b + 1), qc * QC:(qc + 1) * QC], tmpq[:])
                nc.sync.dma_start(
                    kt_sb[32 * b:32 * (b + 1), qc * QC:(qc + 1) * QC], tmpk[:])
            # V: wv-stationary -> V_T [32, KV], then PE-transpose per slab
            vt_sb = tmpp.tile([32, KV], F32, tag="vt", bufs=1, name=f"vt{b}")
            for ck in range(4):
                vt = pj_ps.tile([32, QC], F32, tag="pj", name=f"vt{b}_{ck}")
                for cc in range(2):
                    nc.tensor.matmul(
                        vt[:], w3_sb[:, cc * 96 + 2 * CH:cc * 96 + 3 * CH],
                        xkv[:, cc * KV + ck * QC: cc * KV + (ck + 1) * QC],
                        start=(cc == 0), stop=(cc == 1))
                nc.vector.tensor_copy(vt_sb[:, ck * QC:(ck + 1) * QC], vt[:])
            for t in range(N_SLAB):
                pv = pj_ps.tile([128, CH], F32, tag="pj", name=f"pv{b}_{t}")
                nc.tensor.transpose(pv[:], vt_sb[:, t * 128:(t + 1) * 128],
                                    id32_sb[:])
                col = (b * N_SLAB + t) * 33
                midx = b * N_SLAB + t
                nc.vector.tensor_scalar_mul(
                    vaug_sb[:, col + 1:col + 1 + CH], pv[:],
                    mask_sb[:, midx:midx + 1])
                nc.vector.tensor_copy(vaug_sb[:, col:col + 1], mask_sb[:, midx:midx + 1])

        def attention(qc, pr, bias_tiles):
            b_lo, b_hi = 2 * pr, 2 * pr + 1
            o_lo = o_ps.tile([33, QC], F32, tag="oA", name=f"olo_{qc}_{pr}")
            o_hi = o_ps.tile([33, QC], F32, tag="oB", name=f"ohi_{qc}_{pr}")
            prev = []  # delayed AV ops: (vaug_col_lo, vaug_col_hi, p_tile)

            def emit_av(item, start, stop):
                col_lo, col_hi, p = item
                nc.tensor.matmul(o_lo[:], vaug_sb[:, col_lo:col_lo + 33],
                                 p[:, 0:QC], start=start, stop=stop)
                nc.tensor.matmul(o_hi[:], vaug_sb[:, col_hi:col_hi + 33],
                                 p[:, QC:2 * QC], start=start, stop=stop)

            for t in range(N_SLAB):
                eb = bias_tiles[t][:, (qc % 2) * QC:(qc % 2 + 1) * QC]
                s01 = s_ps.tile([128, 2 * QC], F32, tag="s", name=f"s_{qc}_{pr}_{t}")
                for half, b in ((0, b_lo), (1, b_hi)):
                    nc.tensor.matmul(
                        s01[:, half * QC:(half + 1) * QC],
                        kt_sb[32 * b:32 * (b + 1), t * 128:(t + 1) * 128],
                        qt_sb[32 * b:32 * (b + 1), qc * QC:(qc + 1) * QC],
                        start=True, stop=True, tile_position=(32 * b, 0))
                es = ppool.tile([128, 2 * QC], BF16, tag="es", name=f"es_{qc}_{pr}_{t}")
                nc.scalar.activation(es[:], s01[:], AF.Exp)
                p = ppool.tile([128, 2 * QC], BF16, tag="p", name=f"p_{qc}_{pr}_{t}")
                nc.vector.tensor_mul(p[:, 0:QC], es[:, 0:QC], eb)
                nc.vector.tensor_mul(p[:, QC:2 * QC], es[:, QC:2 * QC], eb)
                prev.append(((b_lo * N_SLAB + t) * 33, (b_hi * N_SLAB + t) * 33, p))
                if len(prev) > 2:
                    emit_av(prev.pop(0), t == 2, False)
            emit_av(prev.pop(0), False, False)
            emit_av(prev.pop(0), False, True)
            # normalization + bf16 store to the exchange buffer
            for b, o_psum in ((b_lo, o_lo), (b_hi, o_hi)):
                recip = normp.tile([1, QC], F32, tag="recip", name=f"rc_{qc}_{b}")
                nc.vector.reciprocal_approx_fast(recip[:], o_psum[0:1, :])
                bc = normp.tile([33, QC], F32, tag="bc", name=f"bc_{qc}_{b}")
                nc.gpsimd.partition_broadcast(bc[:], recip[:])
                ot = normp.tile([33, QC], BF16, tag="ot", name=f"ot_{qc}_{b}")
                nc.vector.tensor_mul(ot[0:32, :], o_psum[0:32, :], bc[0:32, :])
                nc.vector.tensor_mul(ot[32:33, :], o_psum[32:33, :], bc[32:33, :])
                dest = 2 * b + (qc % 2)
                nc.sync.dma_start(ot_send[qc // 2][dest], ot[1:33, :])

        # ---- schedule ----
        bias_q01 = load_bias(0)
        proj(0)
        proj(1)
        attention(0, 0, bias_q01)
        proj(2)
        proj(3)
        attention(0, 1, bias_q01)
        attention(1, 0, bias_q01)
        attention(1, 1, bias_q01)
        nc.gpsimd.collective_compute(
            "AllToAll", mybir.AluOpType.bypass,
            replica_groups=[list(range(N_CORES))],
            ins=[ot_send[0][:]], outs=[ot_recv[0][:]])
        bias_q23 = load_bias(1)
        attention(2, 0, bias_q23)
        attention(2, 1, bias_q23)
        attention(3, 0, bias_q23)
        attention(3, 1, bias_q23)
        nc.gpsimd.collective_compute(
            "AllToAll", mybir.AluOpType.bypass,
            replica_groups=[list(range(N_CORES))],
            ins=[ot_send[1][:]], outs=[ot_recv[1][:]])
        proj_st.close()

        # ---- final projection ----
        with (
            tc.tile_pool(name="finp", bufs=2) as finp,
            tc.tile_pool(name="fin_ps", bufs=2, space="PSUM") as fin_ps,
        ):
            otl = finp.tile([128, 2 * 1024], BF16, tag="otl", bufs=1)
            # otl partition = (slab%4, ch); free col = dc*1024 + ex*512 + i
            for ex in range(2):
                for dc in range(2):
                    nc.sync.dma_start(
                        otl[:, dc * 1024 + ex * QC: dc * 1024 + (ex + 1) * QC],
                        ot_recv[ex][4 * dc:4 * dc + 4])
            for q8 in range(8):
                ex, i = q8 // 4, (q8 % 4) * 128
                fp = fin_ps.tile([128, C_Q], F32, tag="fin", name=f"fin{q8}")
                nc.tensor.matmul(fp[:], ones_sb[0:1, :], bo_sb[:],
                                 start=True, stop=False)
                for dc in range(2):
                    nc.tensor.matmul(
                        fp[:], otl[:, dc * 1024 + ex * QC + i:
                                   dc * 1024 + ex * QC + i + 128],
                        wo_sb[:, dc * C_Q:(dc + 1) * C_Q],
                        start=False, stop=(dc == 1))
                fout = finp.tile([128, C_Q], F32, tag="fout", name=f"fout{q8}")
                nc.vector.tensor_copy(fout[:], fp[:])
                nc.sync.dma_start(out[q8 * 128:(q8 + 1) * 128, :], fout[:])

    nc.compile()
    return nc


def host_inputs(input_q, input_kv, mask, bias, w_q, w_k, w_v, w_o, b_o):
    import ml_dtypes
    xq_t = np.ascontiguousarray(input_q.transpose(0, 2, 1))
    xkv_t = np.ascontiguousarray(input_kv.transpose(0, 2, 1))
    ident32 = np.eye(32, dtype=np.float32)
    mask_v = np.ascontiguousarray(
        mask.reshape(B, KV).reshape(B, N_SLAB, 128).transpose(0, 2, 1)
    ).astype(np.float32)
    ident = np.eye(128, dtype=ml_dtypes.bfloat16)
    ones_b = np.ones((1, 128), dtype=ml_dtypes.bfloat16)
    bo_b = b_o.reshape(1, C_Q).astype(ml_dtypes.bfloat16)
    # wo packed [128, 2*C_Q]: wo_p[p, dc*C_Q + o] = w_o[dc*128 + p, o]
    wo_p = np.ascontiguousarray(
        w_o.reshape(2, 128, C_Q).transpose(1, 0, 2).reshape(128, 2 * C_Q)
    ).astype(ml_dtypes.bfloat16)
    in_maps = []
    for h in range(N_CORES):
        sl = slice(h * CH, (h + 1) * CH)
        # w3 [128, cc-major: wq*scale | wk | wv]
        w3 = np.empty((128, 6 * CH), ml_dtypes.bfloat16)
        for cc in range(2):
            rows = slice(cc * 128, (cc + 1) * 128)
            w3[:, cc * 96:cc * 96 + CH] = w_q[rows, sl] * SCALE
            w3[:, cc * 96 + CH:cc * 96 + 2 * CH] = w_k[rows, sl]
            w3[:, cc * 96 + 2 * CH:cc * 96 + 3 * CH] = w_v[rows, sl]
        in_maps.append({
            "x_qt": xq_t,
            "x_kvt": xkv_t,
            "w3": w3,
            "bias_t": np.ascontiguousarray(bias[0, h].T),
            "mask_p": mask_v,
            "ident": ident,
            "ident32": ident32,
            "ones_bf": ones_b,
            "wo_bf": wo_p,
            "bo_bf": bo_b,
        })
    return in_maps


def unshard(results):
    # core d rows: local r = ex*512 + i  ->  global (d//2, q=(d%2)*512 + ex*1024 + i)
    out = np.empty((B, Q, C_Q), np.float32)
    for d, r in enumerate(results):
        o = r["out"]  # [1024, C_Q]
        b, half = d // 2, d % 2
        for ex in range(2):
            q0 = half * 512 + ex * 1024
            out[b, q0:q0 + 512] = o[ex * 512:(ex + 1) * 512]
    return out.reshape(B, Q, C_Q)


_CACHED_NC = None


def _get_nc():
    global _CACHED_NC
    if _CACHED_NC is None:
        _CACHED_NC = build()
    return _CACHED_NC


def kernel(input_q, input_kv, mask, bias, w_q, w_k, w_v, w_o, b_o,
           trace=False, **trace_kwargs):
    from concourse.bass_utils import run_bass_kernel_spmd
    args = [np.asarray(x, dtype=np.float32) for x in
            (input_q, input_kv, mask, bias, w_q, w_k, w_v, w_o, b_o)]
    in_maps = host_inputs(*args)
    nc = _get_nc()
    res = run_bass_kernel_spmd(nc, in_maps, core_ids=list(range(N_CORES)),
                               trace=trace, **trace_kwargs)
    out = unshard(res.results)
    if trace:
        return out, res
    return out


# revision 12
# speedup vs baseline: 1.1133x; 1.1133x over previous
"""CrossAttentionNoGate, head-sharded over 8 TRN2 cores (v2).

Core h computes head h for all 4 batches:
  Q_T[b] [32,2048] = (x_q[b] @ wq_h*scale).T   (packed at partitions 32b)
  K_T[b] [32,2048] = (x_kv[b] @ wk_h).T
  V_T[b] [32,2048]  -> PE-transposed per 128-slab -> V_aug[b][t] [128,33]
                      = [mask | V*mask]
  per (qc, pr, t): s01 [128,1024] PSUM = [bias+K^TQ(b_lo) | bias+K^TQ(b_hi)]
      (bias added by PE identity-inject for BOTH batches)
  p = exp(s01)  (single ACT op over 2 banks)
  o_both accumulates V_aug^T p for both batches (one PSUM bank, offsets 0/64)
  normalize by denominator row -> bf16 -> DRAM send buffers
Two bf16 AllToAlls (after qc1 and qc3), dest core d = 2b + (qc&1),
column = 512*(qc>>1).  Final: out rows = otl.T @ w_o + b_o  (bf16 matmuls).
"""
from contextlib import ExitStack

import numpy as np

import concourse.bass as bass
import concourse.tile as tile
from concourse import bacc, mybir

F32 = mybir.dt.float32
F32R = mybir.dt.float32r
BF16 = mybir.dt.bfloat16
AF = mybir.ActivationFunctionType

B, Q, KV, C_Q = 4, 2048, 2048, 256
CH = 32
N_CORES = 8
QC = 512
N_QC = Q // QC        # 4
N_SLAB = KV // 128    # 16
SCALE = 1.0 / np.sqrt(CH)
AV_DELAY = 4


def build():
    nc = bacc.Bacc("TRN2", target_bir_lowering=False, debug=False, num_devices=N_CORES)

    x_qT = nc.dram_tensor("x_qt", [B, C_Q, Q], F32, kind="ExternalInput").ap()
    x_kvT = nc.dram_tensor("x_kvt", [B, C_Q, KV], F32, kind="ExternalInput").ap()
    # [cc(2)][wq|wk|wv][CH] packed, wq pre-scaled by 1/sqrt(CH)
    w3 = nc.dram_tensor("w3", [C_Q // 2, 6 * CH], BF16, kind="ExternalInput").ap()
    bias_t = nc.dram_tensor("bias_t", [KV, Q], F32, kind="ExternalInput").ap()
    mask_p = nc.dram_tensor("mask_p", [B, 128, N_SLAB], F32, kind="ExternalInput").ap()
    ident = nc.dram_tensor("ident", [128, 128], BF16, kind="ExternalInput").ap()
    ident32 = nc.dram_tensor("ident32", [32, 32], F32, kind="ExternalInput").ap()
    ones_bf = nc.dram_tensor("ones_bf", [1, 128], BF16, kind="ExternalInput").ap()
    wo_bf = nc.dram_tensor("wo_bf", [C_Q // 2, 2 * C_Q], BF16, kind="ExternalInput").ap()
    bo_bf = nc.dram_tensor("bo_bf", [1, C_Q], BF16, kind="ExternalInput").ap()

    out = nc.dram_tensor("out", [B * Q // N_CORES, C_Q], F32, kind="ExternalOutput").ap()

    with tile.TileContext(nc) as tc, ExitStack() as st:
        constp = st.enter_context(tc.tile_pool(name="const", bufs=1))
        persist = st.enter_context(tc.tile_pool(name="persist", bufs=1))
        dramp = st.enter_context(tc.tile_pool(name="dram", bufs=1, space="DRAM"))
        biasp = st.enter_context(tc.tile_pool(name="biasp", bufs=4))
        ebp = st.enter_context(tc.tile_pool(name="ebp", bufs=16))
        s_ps = st.enter_context(tc.tile_pool(name="s_ps", bufs=2, space="PSUM"))
        o_ps = st.enter_context(tc.tile_pool(name="o_ps", bufs=1, space="PSUM"))
        ppool = st.enter_context(tc.tile_pool(name="ppool", bufs=3))
        normp = st.enter_context(tc.tile_pool(name="normp", bufs=2))
        proj_st = st.enter_context(ExitStack())
        xpool = proj_st.enter_context(tc.tile_pool(name="xpool", bufs=2))
        tmpp = proj_st.enter_context(tc.tile_pool(name="tmpp", bufs=2))
        pj_ps = proj_st.enter_context(tc.tile_pool(name="pj_ps", bufs=2, space="PSUM"))

        # ---- constants ----
        id_sb = constp.tile([128, 128], BF16)
        nc.sync.dma_start(id_sb[:], ident[:])
        id32_sb = constp.tile([32, 32], F32)
        nc.sync.dma_start(id32_sb[:], ident32[:])
        w3_sb = constp.tile([128, 6 * CH], BF16)
        nc.sync.dma_start(w3_sb[:], w3[:])
        mask_sb = constp.tile([128, B * N_SLAB], F32)
        for b in range(B):
            nc.sync.dma_start(mask_sb[:, b * N_SLAB:(b + 1) * N_SLAB], mask_p[b])
        ones_sb = constp.tile([1, 128], BF16)
        nc.sync.dma_start(ones_sb[:], ones_bf[:])
        wo_sb = constp.tile([128, 2 * C_Q], BF16)
        nc.sync.dma_start(wo_sb[:], wo_bf[:])
        bo_sb = constp.tile([1, C_Q], BF16)
        nc.sync.dma_start(bo_sb[:], bo_bf[:])

        # persistent activations
        qt_sb = persist.tile([128, Q], BF16)
        kt_sb = persist.tile([128, Q], BF16)
        vaug_sb = persist.tile([128, B * N_SLAB * 33], BF16)
        # split exchange buffers: ex 0 covers qc0+qc1, ex 1 covers qc2+qc3
        ot_send = [dramp.tile([N_CORES, CH, QC], BF16, name=f"ot_send{e}")
                   for e in range(2)]
        ot_recv = [dramp.tile([N_CORES, CH, QC], BF16, name=f"ot_recv{e}")
                   for e in range(2)]

        def load_bias(qcp):
            tiles = []
            for t in range(N_SLAB):
                bt = biasp.tile([128, 2 * QC], BF16, tag="bias",
                                name=f"bias_{qcp}_{t}")
                nc.gpsimd.dma_start(
                    bt[:], bias_t[t * 128:(t + 1) * 128,
                                  qcp * 2 * QC:(qcp + 1) * 2 * QC])
                eb = ebp.tile([128, 2 * QC], BF16, tag="eb", name=f"eb_{qcp}_{t}")
                nc.scalar.activation(eb[:], bt[:], AF.Exp)
                tiles.append(eb)
            return tiles

        def proj(b):
            xq = xpool.tile([128, 2 * Q], BF16, tag="xq", name=f"xq{b}")
            xkv = xpool.tile([128, 2 * KV], BF16, tag="xkv", name=f"xkv{b}")
            for cc in range(2):
                nc.gpsimd.dma_start(xq[:, cc * Q:(cc + 1) * Q],
                                    x_qT[b, cc * 128:(cc + 1) * 128, :])
                nc.gpsimd.dma_start(xkv[:, cc * KV:(cc + 1) * KV],
                                    x_kvT[b, cc * 128:(cc + 1) * 128, :])
            # q/k projections -> [32, 512] psum stripes -> qt/kt rows 32b
            for qc in range(N_QC):
                pq = pj_ps.tile([32, QC], F32, tag="pj", name=f"pq{b}_{qc}")
                pk = pj_ps.tile([32, QC], F32, tag="pj", name=f"pk{b}_{qc}")
                for cc in range(2):
                    nc.tensor.matmul(
                        pq[:], w3_sb[:, cc * 96:cc * 96 + CH],
                        xq[:, cc * Q + qc * QC: cc * Q + (qc + 1) * QC],
                        start=(cc == 0), stop=(cc == 1))
                    nc.tensor.matmul(
                        pk[:], w3_sb[:, cc * 96 + CH:cc * 96 + 2 * CH],
                        xkv[:, cc * KV + qc * QC: cc * KV + (qc + 1) * QC],
                        start=(cc == 0), stop=(cc == 1))
                tmpq = tmpp.tile([32, QC], BF16, tag="tq", name=f"tq{b}_{qc}")
                tmpk = tmpp.tile([32, QC], BF16, tag="tk", name=f"tk{b}_{qc}")
                nc.vector.tensor_copy(tmpq[:], pq[:])
                nc.vector.tensor_copy(tmpk[:], pk[:])
                nc.sync.dma_start(
                    qt_sb[32 * b:32 * (b + 1), qc * QC:(qc + 1) * QC], tmpq[:])
                nc.sync.dma_start(
                    kt_sb[32 * b:32 * (b + 1), qc * QC:(qc + 1) * QC], tmpk[:])
            # V: wv-stationary -> V_T [32, KV], then PE-transpose per slab
            vt_sb = tmpp.tile([32, KV], F32, tag="vt", bufs=1, name=f"vt{b}")
            for ck in range(4):
                vt = pj_ps.tile([32, QC], F32, tag="pj", name=f"vt{b}_{ck}")
                for cc in range(2):
                    nc.tensor.matmul(
                        vt[:], w3_sb[:, cc * 96 + 2 * CH:cc * 96 + 3 * CH],
                        xkv[:, cc * KV + ck * QC: cc * KV + (ck + 1) * QC],
                        start=(cc == 0), stop=(cc == 1))
                nc.vector.tensor_copy(vt_sb[:, ck * QC:(ck + 1) * QC], vt[:])
            for t in range(N_SLAB):
                pv = pj_ps.tile([128, CH], F32, tag="pj", name=f"pv{b}_{t}")
                nc.tensor.transpose(pv[:], vt_sb[:, t * 128:(t + 1) * 128],
                                    id32_sb[:])
                col = (b * N_SLAB + t) * 33
                midx = b * N_SLAB + t
                nc.vector.tensor_scalar_mul(
                    vaug_sb[:, col + 1:col + 1 + CH], pv[:],
                    mask_sb[:, midx:midx + 1])
                nc.vector.tensor_copy(vaug_sb[:, col:col + 1], mask_sb[:, midx:midx + 1])

        def attention(qc, pr, bias_tiles):
            b_lo, b_hi = 2 * pr, 2 * pr + 1
            o_lo = o_ps.tile([33, QC], F32, tag="oA", name=f"olo_{qc}_{pr}")
            o_hi = o_ps.tile([33, QC], F32, tag="oB", name=f"ohi_{qc}_{pr}")
            prev = []  # delayed AV ops: (vaug_col_lo, vaug_col_hi, p_tile)

            def emit_av(item, start, stop):
                col_lo, col_hi, p = item
                nc.tensor.matmul(o_lo[:], vaug_sb[:, col_lo:col_lo + 33],
                                 p[:, 0:QC], start=start, stop=stop)
                nc.tensor.matmul(o_hi[:], vaug_sb[:, col_hi:col_hi + 33],
                                 p[:, QC:2 * QC], start=start, stop=stop)

            for t in range(N_SLAB):
                eb = bias_tiles[t][:, (qc % 2) * QC:(qc % 2 + 1) * QC]
                s01 = s_ps.tile([128, 2 * QC], F32, tag="s", name=f"s_{qc}_{pr}_{t}")
                for half, b in ((0, b_lo), (1, b_hi)):
                    nc.tensor.matmul(
                        s01[:, half * QC:(half + 1) * QC],
                        kt_sb[32 * b:32 * (b + 1), t * 128:(t + 1) * 128],
                        qt_sb[32 * b:32 * (b + 1), qc * QC:(qc + 1) * QC],
                        start=True, stop=True, tile_position=(32 * b, 0))
                es = ppool.tile([128, 2 * QC], BF16, tag="es", bufs=6,
                                name=f"es_{qc}_{pr}_{t}")
                nc.scalar.activation(es[:], s01[:], AF.Exp)
                p = ppool.tile([128, 2 * QC], BF16, tag="p", bufs=6,
                               name=f"p_{qc}_{pr}_{t}")
                nc.vector.tensor_mul(p[:, 0:QC], es[:, 0:QC], eb)
                nc.vector.tensor_mul(p[:, QC:2 * QC], es[:, QC:2 * QC], eb)
                prev.append(((b_lo * N_SLAB + t) * 33, (b_hi * N_SLAB + t) * 33, p))
                if len(prev) > AV_DELAY:
                    emit_av(prev.pop(0), t == AV_DELAY, False)
            while len(prev) > 1:
                emit_av(prev.pop(0), False, False)
            emit_av(prev.pop(0), False, True)
            # normalization + bf16 store to the exchange buffer
            for b, o_psum in ((b_lo, o_lo), (b_hi, o_hi)):
                recip = normp.tile([1, QC], F32, tag="recip", name=f"rc_{qc}_{b}")
                nc.vector.reciprocal_approx_fast(recip[:], o_psum[0:1, :])
                bc = normp.tile([33, QC], F32, tag="bc", name=f"bc_{qc}_{b}")
                nc.gpsimd.partition_broadcast(bc[:], recip[:])
                ot = normp.tile([33, QC], BF16, tag="ot", name=f"ot_{qc}_{b}")
                nc.vector.tensor_mul(ot[0:32, :], o_psum[0:32, :], bc[0:32, :])
                nc.vector.tensor_mul(ot[32:33, :], o_psum[32:33, :], bc[32:33, :])
                dest = 2 * b + (qc % 2)
                nc.sync.dma_start(ot_send[qc // 2][dest], ot[1:33, :])

        # ---- schedule ----
        bias_q01 = load_bias(0)
        proj(0)
        proj(1)
        attention(0, 0, bias_q01)
        proj(2)
        proj(3)
        attention(0, 1, bias_q01)
        attention(1, 0, bias_q01)
        attention(1, 1, bias_q01)
        nc.gpsimd.collective_compute(
            "AllToAll", mybir.AluOpType.bypass,
            replica_groups=[list(range(N_CORES))],
            ins=[ot_send[0][:]], outs=[ot_recv[0][:]])
        bias_q23 = load_bias(1)
        attention(2, 0, bias_q23)
        attention(2, 1, bias_q23)
        attention(3, 0, bias_q23)
        attention(3, 1, bias_q23)
        nc.gpsimd.collective_compute(
            "AllToAll", mybir.AluOpType.bypass,
            replica_groups=[list(range(N_CORES))],
            ins=[ot_send[1][:]], outs=[ot_recv[1][:]])
        proj_st.close()

        # ---- final projection ----
        with (
            tc.tile_pool(name="finp", bufs=2) as finp,
            tc.tile_pool(name="fin_ps", bufs=2, space="PSUM") as fin_ps,
        ):
            otl = finp.tile([128, 2 * 1024], BF16, tag="otl", bufs=1)
            # otl partition = (slab%4, ch); free col = dc*1024 + ex*512 + i
            for ex in range(2):
                for dc in range(2):
                    nc.sync.dma_start(
                        otl[:, dc * 1024 + ex * QC: dc * 1024 + (ex + 1) * QC],
                        ot_recv[ex][4 * dc:4 * dc + 4])
            for q8 in range(8):
                ex, i = q8 // 4, (q8 % 4) * 128
                fp = fin_ps.tile([128, C_Q], F32, tag="fin", name=f"fin{q8}")
                nc.tensor.matmul(fp[:], ones_sb[0:1, :], bo_sb[:],
                                 start=True, stop=False)
                for dc in range(2):
                    nc.tensor.matmul(
                        fp[:], otl[:, dc * 1024 + ex * QC + i:
                                   dc * 1024 + ex * QC + i + 128],
                        wo_sb[:, dc * C_Q:(dc + 1) * C_Q],
                        start=False, stop=(dc == 1))
                fout = finp.tile([128, C_Q], F32, tag="fout", name=f"fout{q8}")
                nc.vector.tensor_copy(fout[:], fp[:])
                nc.sync.dma_start(out[q8 * 128:(q8 + 1) * 128, :], fout[:])

    nc.compile()
    return nc


def host_inputs(input_q, input_kv, mask, bias, w_q, w_k, w_v, w_o, b_o):
    import ml_dtypes
    xq_t = np.ascontiguousarray(input_q.transpose(0, 2, 1))
    xkv_t = np.ascontiguousarray(input_kv.transpose(0, 2, 1))
    ident32 = np.eye(32, dtype=np.float32)
    mask_v = np.ascontiguousarray(
        mask.reshape(B, KV).reshape(B, N_SLAB, 128).transpose(0, 2, 1)
    ).astype(np.float32)
    ident = np.eye(128, dtype=ml_dtypes.bfloat16)
    ones_b = np.ones((1, 128), dtype=ml_dtypes.bfloat16)
    bo_b = b_o.reshape(1, C_Q).astype(ml_dtypes.bfloat16)
    # wo packed [128, 2*C_Q]: wo_p[p, dc*C_Q + o] = w_o[dc*128 + p, o]
    wo_p = np.ascontiguousarray(
        w_o.reshape(2, 128, C_Q).transpose(1, 0, 2).reshape(128, 2 * C_Q)
    ).astype(ml_dtypes.bfloat16)
    in_maps = []
    for h in range(N_CORES):
        sl = slice(h * CH, (h + 1) * CH)
        # w3 [128, cc-major: wq*scale | wk | wv]
        w3 = np.empty((128, 6 * CH), ml_dtypes.bfloat16)
        for cc in range(2):
            rows = slice(cc * 128, (cc + 1) * 128)
            w3[:, cc * 96:cc * 96 + CH] = w_q[rows, sl] * SCALE
            w3[:, cc * 96 + CH:cc * 96 + 2 * CH] = w_k[rows, sl]
            w3[:, cc * 96 + 2 * CH:cc * 96 + 3 * CH] = w_v[rows, sl]
        in_maps.append({
            "x_qt": xq_t,
            "x_kvt": xkv_t,
            "w3": w3,
            "bias_t": np.ascontiguousarray(bias[0, h].T),
            "mask_p": mask_v,
            "ident": ident,
            "ident32": ident32,
            "ones_bf": ones_b,
            "wo_bf": wo_p,
            "bo_bf": bo_b,
        })
    return in_maps


def unshard(results):
    # core d rows: local r = ex*512 + i  ->  global (d//2, q=(d%2)*512 + ex*1024 + i)
    out = np.empty((B, Q, C_Q), np.float32)
    for d, r in enumerate(results):
        o = r["out"]  # [1024, C_Q]
        b, half = d // 2, d % 2
        for ex in range(2):
            q0 = half * 512 + ex * 1024
            out[b, q0:q0 + 512] = o[ex * 512:(ex + 1) * 512]
    return out.reshape(B, Q, C_Q)


_CACHED_NC = None


def _get_nc():
    global _CACHED_NC
    if _CACHED_NC is None:
        _CACHED_NC = build()
    return _CACHED_NC


def kernel(input_q, input_kv, mask, bias, w_q, w_k, w_v, w_o, b_o,
           trace=False, **trace_kwargs):
    from concourse.bass_utils import run_bass_kernel_spmd
    args = [np.asarray(x, dtype=np.float32) for x in
            (input_q, input_kv, mask, bias, w_q, w_k, w_v, w_o, b_o)]
    in_maps = host_inputs(*args)
    nc = _get_nc()
    res = run_bass_kernel_spmd(nc, in_maps, core_ids=list(range(N_CORES)),
                               trace=trace, **trace_kwargs)
    out = unshard(res.results)
    if trace:
        return out, res
    return out
